# revision 5
# baseline (speedup 1.0000x reference)
"""SEIR Euler integration kernel for 8 TRN2 NeuronCores.

Shards the batch axis (B=32768) across 8 cores (4096 each); every core runs
the full 1024-step Euler scan on its shard and streams the trajectory to DRAM.

Per-core state layout: one SBUF "staging" tile per K-step block, shaped
[128 partitions, K*128] fp32, where column k*128 + g*4 + c holds compartment
c (S,E,I,R) of batch element b = p*32 + g at step t0+k.  This matches the
output DRAM layout (t*B + b, 4) exactly, so block stores are contiguous
512B-per-partition DMAs.  Compute ops read/write strided (stride-4) views of
the staging tiles directly, so the staging tiles ARE the state.

Toolchain constraint: this container's walrus build rejects instructions
carrying >2 semaphore waits ("Too many sync wait commands") and never splits
them.  The Tile tail-drain waits on one sem per engine + DMA lane used, so
the kernel must use exactly one compute engine (DVE) and one DMA sem lane.
We patch Tile's DMA-lane constants to 1 and issue every DMA on nc.sync
(HWDGE).
"""

import sys

sys.path.insert(0, "/opt/trn_rl_repo")

import numpy as np

import concourse.bass as bass
import concourse.tile as tile
import concourse.tile_sem_assignment as _tsa
from concourse import mybir
from concourse.bass_utils import run_bass_kernel_spmd

# One DMA-completion sem lane each for HWDGE/SWDGE: keeps every instruction
# (and the auto-emitted tail drain) within walrus' 2-sync-wait limit.
_tsa.NUM_HWDGE_SEMS = 1
_tsa.NUM_SWDGE_GLOBAL_SEMS = 1

T = 1024
B = 32768
NCORES = 8
BS = B // NCORES  # 4096 batch elements per core
P = 128  # SBUF partitions
G = BS // P  # 32 batch elements per partition
C = 4  # compartments S,E,I,R
FREE = G * C  # 128 floats per step per partition
K = 16  # steps per DMA block
NBLK = T // K

TRACE = False  # test.py flips this to profile

f32 = mybir.dt.float32
mult = mybir.AluOpType.mult
add = mybir.AluOpType.add


def _build():
    nc = bass.Bass(trn_type="TRN2")
    init = nc.dram_tensor("initial", [C, BS], f32, kind="ExternalInput")
    beta = nc.dram_tensor("beta", [1], f32, kind="ExternalInput")
    gamma = nc.dram_tensor("gamma", [1], f32, kind="ExternalInput")
    sigma = nc.dram_tensor("sigma", [1], f32, kind="ExternalInput")
    out = nc.dram_tensor("out", [T, P, FREE], f32, kind="ExternalOutput")

    with tile.TileContext(nc) as tc:
        with (
            tc.tile_pool(name="consts", bufs=1) as consts,
            tc.tile_pool(name="stage", bufs=3) as stagep,
            tc.tile_pool(name="scratch", bufs=4) as scratch,
        ):
            # ---- broadcast the three rate scalars to all partitions ----
            bt = consts.tile([P, 1], f32, tag="bt")
            gt = consts.tile([P, 1], f32, tag="gt")
            st = consts.tile([P, 1], f32, tag="st")
            for dst, src in ((bt, beta), (gt, gamma), (st, sigma)):
                src_ap = src[:]
                bcast = bass.AP(
                    tensor=src_ap.tensor,
                    offset=src_ap.offset,
                    ap=[[0, P], [1, 1]],
                )
                nc.sync.dma_start(out=dst[:, :], in_=bcast)

            # derived per-partition scalars (h = 0.5 Euler step)
            c1t = consts.tile([P, 1], f32, tag="c1")  # 0.5*beta
            c2t = consts.tile([P, 1], f32, tag="c2")  # 0.5*sigma
            c3t = consts.tile([P, 1], f32, tag="c3")  # 0.5*gamma
            a2t = consts.tile([P, 1], f32, tag="a2")  # 1 - 0.5*sigma
            a3t = consts.tile([P, 1], f32, tag="a3")  # 1 - 0.5*gamma
            nc.vector.tensor_scalar_mul(c1t[:, :], bt[:, :], 0.5)
            nc.vector.tensor_scalar_mul(c2t[:, :], st[:, :], 0.5)
            nc.vector.tensor_scalar_mul(c3t[:, :], gt[:, :], 0.5)
            nc.vector.tensor_scalar(a2t[:, :], st[:, :], -0.5, 1.0, mult, add)
            nc.vector.tensor_scalar(a3t[:, :], gt[:, :], -0.5, 1.0, mult, add)
            c1 = c1t[:, 0:1]
            c2 = c2t[:, 0:1]
            c3 = c3t[:, 0:1]
            a2 = a2t[:, 0:1]
            a3 = a3t[:, 0:1]

            # ---- initial state into block 0, step column 0 ----
            # One DMA into a temp tile, then one DVE copy to interleave, so
            # downstream compute depends on a single engine-order chain.
            cur = stagep.tile([P, K * FREE], f32, tag="stage")
            r = cur[:, :].rearrange("p (k g c) -> p k c g", k=K, g=G, c=C)
            tmp0 = consts.tile([P, FREE], f32, tag="init_tmp")
            nc.sync.dma_start(
                out=tmp0[:, :].rearrange("p (c g) -> p c g", c=C),
                in_=init[:, :].rearrange("c (p g) -> p c g", p=P),
            )
            nc.vector.tensor_copy(
                out=r[:, 0, :, :],
                in_=tmp0[:, :].rearrange("p (c g) -> p c g", c=C),
            )

            prev_r, prev_k = r, 0
            for blk in range(NBLK):
                if blk > 0:
                    cur = stagep.tile([P, K * FREE], f32, tag="stage")
                    r = cur[:, :].rearrange("p (k g c) -> p k c g", k=K, g=G, c=C)
                ks = range(1, K) if blk == 0 else range(K)
                for k in ks:
                    Sp = prev_r[:, prev_k, 0, :]
                    Ep = prev_r[:, prev_k, 1, :]
                    Ip = prev_r[:, prev_k, 2, :]
                    Rp = prev_r[:, prev_k, 3, :]
                    w = scratch.tile([P, G], f32, tag="w")
                    d1 = scratch.tile([P, G], f32, tag="d1")
                    # W = c2*E
                    nc.vector.tensor_scalar_mul(w[:, :], Ep, c2)
                    # D1 = c1*S*I
                    nc.vector.scalar_tensor_tensor(d1[:, :], Sp, c1, Ip, mult, mult)
                    # Sn = S - D1
                    nc.vector.tensor_sub(r[:, k, 0, :], Sp, d1[:, :])
                    # En = a2*E + D1
                    nc.vector.scalar_tensor_tensor(
                        r[:, k, 1, :], Ep, a2, d1[:, :], mult, add
                    )
                    # Rn = c3*I + R
                    nc.vector.scalar_tensor_tensor(r[:, k, 3, :], Ip, c3, Rp, mult, add)
                    # In = a3*I + W
                    nc.vector.scalar_tensor_tensor(
                        r[:, k, 2, :], Ip, a3, w[:, :], mult, add
                    )
                    prev_r, prev_k = r, k
                # store the block: DRAM [K,P,FREE] k-major, iterate p-outer
                dview = out[blk * K : (blk + 1) * K, :, :].rearrange("k p f -> p k f")
                sview = cur[:, :].rearrange("p (k f) -> p k f", k=K)
                nc.sync.dma_start(out=dview, in_=sview)

    # This container's walrus build fits only ONE sync wait per instruction
    # and never splits extras into separate waits, so legalize manually:
    #
    # 1. DMACopy: drop the DMAHW lane-ordering wait when a data (DVE) wait is
    #    present.  All HWDGE DMAs issue from one SP FIFO ring and each incs
    #    the shared lane sem by exactly 16, so "sem >= 16*N" still implies
    #    the first N DMAs completed; the DMAs have no data deps on each
    #    other (stage-tile WAR is enforced on the compute side).
    # 2. Any instruction: drop 'sem-ge' waits on its OWN engine's sem.  The
    #    engine executes its stream in order, so those waits are trivially
    #    satisfied (they only exist because Tile merges tile-WAR deps into
    #    the engine chain).
    # 3. The pre-barrier Drain waits on [DVE, DMAHW0]; keep only DMAHW0.
    #    The final block-store DMA waits on the full DVE chain, so DMA-lane
    #    completion transitively implies DVE completion.
    for bb in nc.m.functions[0].blocks:
        for ins in bb.instructions:
            si = ins.sync_info
            if si is None:
                continue
            ow = si.on_wait
            if not ow or len(ow) < 2:
                continue
            kind = ins.__class__.__name__
            eng = str(ins.engine).rsplit(".", 1)[-1]
            if kind == "InstDMACopy":
                new_w = [
                    w
                    for w in ow
                    if not (
                        w.ant_name.startswith("DMAHW")
                        or w.ant_name.startswith("DMASW")
                    )
                ]
            elif kind == "InstDrain":
                dma_w = [w for w in ow if w.ant_name.startswith("DMA")]
                new_w = dma_w[-1:] if dma_w else ow[-1:]
            else:
                new_w = [
                    w
                    for w in ow
                    if not (
                        w.wait_mode == "sem-ge-imm"
                        and w.ant_name.split("_")[0] == eng
                    )
                ]
            if len(new_w) < len(ow):
                si.on_wait = new_w
                ins.sync_info = si
    return nc


_nc = None


def kernel(initial, beta, gamma, sigma, t):
    global _nc
    assert int(t) == T
    initial = np.ascontiguousarray(np.asarray(initial, dtype=np.float32))
    beta = np.asarray(beta, dtype=np.float32).reshape(1)
    gamma = np.asarray(gamma, dtype=np.float32).reshape(1)
    sigma = np.asarray(sigma, dtype=np.float32).reshape(1)
    assert initial.shape == (C, B)

    if _nc is None:
        _nc = _build()

    in_maps = []
    for i in range(NCORES):
        shard = np.ascontiguousarray(initial[:, i * BS : (i + 1) * BS])
        in_maps.append(
            {"initial": shard, "beta": beta, "gamma": gamma, "sigma": sigma}
        )

    res = run_bass_kernel_spmd(
        _nc, in_maps, core_ids=list(range(NCORES)), trace=TRACE
    )
    if TRACE and res.exec_time_ns is not None:
        print(f"HW exec time: {res.exec_time_ns} ns")

    full = np.empty((T, NCORES, BS, C), dtype=np.float32)
    for i in range(NCORES):
        # [T, P, FREE] -> (T, P, G, C) -> (T, BS, C); b_local = p*G + g
        full[:, i] = res.results[i]["out"].reshape(T, P, G, C).reshape(T, BS, C)
    return full.reshape(T * B, C)


if __name__ == "__main__":
    rng = np.random.default_rng(0)
    ini = rng.random((C, B), dtype=np.float32)
    be, ga, si = (rng.random(1, dtype=np.float32) for _ in range(3))
    outv = kernel(ini, be, ga, si, T)
    print("ran, out shape", outv.shape, outv[:4])
